# revision 33
# baseline (speedup 1.0000x reference)
"""Teacher-forced decoder LSTM on 8 TRN2 NeuronCores.

Problem: B=256, T=32, V=10000, E=H=512 (fp32 in/out).
  step s in 0..30: x = embed[caps[:, s]]
                   gates = x@W_ih.T + h@W_hh.T + b     (i,f,g,o)
                   c = sig(f)*c + sig(i)*tanh(g); h = sig(o)*tanh(c)
                   out[s+1] = h@W_lin.T + b_lin
  out[0] = 0.  Output [T, B, V].

Sharding: data-parallel over batch, B_local=32 per core.

Key idea vs the straightforward layout: keep the whole recurrence in
TRANSPOSED space. Gates are computed as gatesT[4H, B_local] via
out[128,32] = W_chunk.T @ hT_chunk matmuls, so the PE moving dimension
is the batch (32) instead of the gate dim (512): per-step PE cost drops
~4x and the cell update produces hT directly in the layout that both
the next step's matmuls and the final logits GEMM consume - no per-step
transposes at all. All matmul operands are bf16 (1 cycle/row at any
moving size); psum accumulation stays fp32 and the cell state c is fp32.

Per step: 64 h-side MMs (N=32) on the critical path; 64 x-side MMs +
1 bias MM (N=512, via a block-indicator rhs) pre-accumulated into one
of 6 rotating psum banks several steps ahead; gate blocks are ordered
[g,i,f,o] (host-permuted weights) so tanh(g) can start early. One
500-vocab-column logits chunk is emitted per step into recurrence gaps.
Phase 3 streams W_lin.T in bf16 super-chunks; logits are stored bf16
and widened to fp32 on the host.
"""
import numpy as np

B_FULL, T, V, E, H = 256, 32, 10000, 512, 512
NCORES = 8
BL = B_FULL // NCORES          # 32 batch rows per core
S = T - 1                      # 31 recurrent steps
M_TOK = S * BL                 # 992 token rows per core (t-major)
NMT = (M_TOK + 127) // 128     # 8 m-tiles (last has 96 rows)
G4 = 4 * H                     # 2048 gate dims
CW = 2000                      # vocab super-chunk width
NSUP = V // CW                 # 5 super-chunks
EC = 500                       # emit chunk width (CW // 4)

_CACHE = {}


def _build():
    import concourse.bacc as bacc
    import concourse.mybir as mybir
    from concourse.tile import TileContext
    import concourse.bass as bass

    f32 = mybir.dt.float32
    bf16 = mybir.dt.bfloat16
    i32 = mybir.dt.int32
    SIG = mybir.ActivationFunctionType.Sigmoid
    TANH = mybir.ActivationFunctionType.Tanh
    ADD = mybir.AluOpType.add
    MUL = mybir.AluOpType.mult

    nc = bacc.Bacc()

    emb_d = nc.dram_tensor("emb", [V, E], bf16, kind="ExternalInput")
    # wihT/whhT pre-arranged on host to [128, 4k x 2048]: k-chunk k at free
    # [2048k:2048(k+1)], gate blocks inside permuted to [g,i,f,o] order.
    wihT_d = nc.dram_tensor("wihT", [128, 4 * G4], bf16, kind="ExternalInput")
    whhT_d = nc.dram_tensor("whhT", [128, 4 * G4], bf16, kind="ExternalInput")
    biasblk_d = nc.dram_tensor("biasblk", [128, 512], f32, kind="ExternalInput")
    h0T_d = nc.dram_tensor("h0T", [128, 128], bf16, kind="ExternalInput")
    tok_d = nc.dram_tensor("tok", [128, NMT], i32, kind="ExternalInput")
    # host-transposed embeddings for m-tiles 0/1 (startup critical path);
    # m-tiles 2..7 are gathered+transposed on device during the recurrence
    xt01_d = nc.dram_tensor("xt01", [128, 1024], bf16, kind="ExternalInput")
    warm_d = nc.dram_tensor("warm", [128, 512], bf16, kind="ExternalInput")
    # wlinT pre-arranged to [128, 4k x 10000]: k-chunk k at [10000k:...]
    wlinT_d = nc.dram_tensor("wlinT", [128, 4 * V], bf16, kind="ExternalInput")
    blin_d = nc.dram_tensor("blin", [128, V], bf16, kind="ExternalInput")
    out_d = nc.dram_tensor("out", [M_TOK, V], bf16, kind="ExternalOutput")

    with TileContext(nc) as tc:
        with tc.tile_pool(name="const", bufs=1) as cp, \
             tc.tile_pool(name="state", bufs=1) as st, \
             tc.tile_pool(name="xst", bufs=2) as xst, \
             tc.tile_pool(name="wlp", bufs=2) as wlp, \
             tc.tile_pool(name="stg", bufs=4) as stp, \
             tc.tile_pool(name="rps", bufs=6, space="PSUM") as rps, \
             tc.tile_pool(name="p3ps", bufs=2, space="PSUM") as p3ps:

            # ---------- constant loads, spread across queues ----------
            # Startup critical path: x(0) needs xt01 + bias16/sel16 + wihT;
            # h(0) additionally needs whhT + h0T. Ws are split in quarters
            # across all 4 DMA-capable queues so each is resident ~2us after
            # its loads start.
            QW = G4  # quarter width of the [128, 4*G4] layout
            wihT = cp.tile([128, 4 * G4], bf16, tag="wihT")
            whhT = cp.tile([128, 4 * G4], bf16, tag="whhT")
            tok_sb = cp.tile([128, NMT], i32, tag="tok_sb")
            biasblk = cp.tile([128, 512], f32, tag="biasblk")
            h0T = cp.tile([128, 128], bf16, tag="h0T")

            # xt[m]: transposed gathered embeddings for m-tile m,
            # E-chunk k at [128k:128(k+1)], token j at col j (4 steps x 32).
            xt = [st.tile([128, 512], bf16, tag=f"xt{m}", name=f"xt{m}")
                  for m in range(NMT)]

            def wq(w_sb, w_d, q, eng):
                eng.dma_start(out=w_sb[:, QW * q:QW * (q + 1)],
                              in_=w_d[:, QW * q:QW * (q + 1)])

            # SP queue: a small warm tile first, then xt for m-tiles 0/1
            warmm = cp.tile([128, 512], bf16, tag="warmm")
            nc.sync.dma_start(out=warmm[:], in_=warm_d[:])
            nc.sync.dma_start(out=xt[0][:], in_=xt01_d[:, 0:512])
            nc.sync.dma_start(out=xt[1][:], in_=xt01_d[:, 512:1024])
            # PE p-state warmup: ~2.5us of dummy matmuls starting ~2.4us in
            # (as soon as warmm lands) so the PE ramp (3us after first MM)
            # completes right as the real recurrence MMs become ready.
            # Uses a pl-tag psum bank (idle until the first logits emit).
            pw = p3ps.tile([128, 512], f32, tag="pl", name="plwarm")
            for d in range(5):
                nc.tensor.matmul(out=pw[0:128, :], lhsT=warmm[:, 0:128],
                                 rhs=warmm[:, 0:512], start=(d == 0),
                                 stop=(d == 4), skip_group_check=True)
            wq(wihT, wihT_d, 0, nc.sync)
            wq(whhT, whhT_d, 0, nc.sync)
            wq(wihT, wihT_d, 3, nc.sync)
            # ACT queue
            wq(wihT, wihT_d, 1, nc.scalar)
            wq(whhT, whhT_d, 1, nc.scalar)
            wq(whhT, whhT_d, 3, nc.scalar)
            # Pool queue
            nc.gpsimd.dma_start(out=tok_sb[:], in_=tok_d[:])
            nc.gpsimd.dma_start(out=biasblk[:], in_=biasblk_d[:])
            nc.gpsimd.dma_start(out=h0T[:], in_=h0T_d[:])
            wq(wihT, wihT_d, 2, nc.gpsimd)
            wq(whhT, whhT_d, 2, nc.gpsimd)
            # lower-priority loads (behind the startup chain)
            wl0 = wlp.tile([128, 4 * CW], bf16, tag="wl", name="wl0")
            for k in range(4):
                nc.scalar.dma_start(out=wl0[:, CW * k:CW * (k + 1)],
                                    in_=wlinT_d[:, V * k:V * k + CW])

            # ---------- state ----------
            # h_allT: transposed hidden states, chunk k at [992k:992(k+1)],
            # step s at cols 32s within each chunk. bf16; rhs of recurrence
            # MMs and lhsT of phase-3 MMs.
            h_allT = st.tile([128, 4 * M_TOK], bf16, tag="h_allT")
            cT = st.tile([128, 128], f32, tag="cT")
            nc.vector.memset(cT[:], 0.0)
            act_sb = st.tile([128, 512], f32, tag="act_sb")  # g|i|f|o blocks
            t1 = st.tile([128, 128], f32, tag="t1")
            t2 = st.tile([128, 128], f32, tag="t2")
            th = st.tile([128, 128], f32, tag="th")

            def gather(m):
                rows = min(128, M_TOK - 128 * m)
                gx = xst.tile([128, 512], bf16, tag="gx", name=f"gx{m}")
                nc.gpsimd.indirect_dma_start(
                    out=gx[0:rows, :], out_offset=None, in_=emb_d[:],
                    in_offset=bass.IndirectOffsetOnAxis(
                        ap=tok_sb[0:rows, m:m + 1], axis=0))
                # single chunked-transpose DMA: out[p, k, j] = gx[j, 128k+p]
                nc.sync.dma_start_transpose(
                    out=xt[m][:].rearrange("p (k j) -> p k j", k=4)[:, :, 0:rows],
                    in_=gx[0:rows, :])

            blin_sb = cp.tile([128, V], bf16, tag="blin_sb")
            nc.sync.dma_start(out=blin_sb[:], in_=blin_d[:])

            # ---------- recurrence helpers ----------
            pgs = {}

            def emit_x(s):
                """Bias init + x-side gate MMs for step s into a fresh psum
                bank. The bias is written by a Pool copy (not a PE matmul);
                all MMs then accumulate with start=False. Safe because every
                bank cycle writes all 512 columns, so no pending-zero bits
                survive from the previous user of the bank."""
                m, a = divmod(s, 4)
                pg = rps.tile([128, 512], f32, tag="pg", name=f"pg{s}")
                pgs[s] = pg
                nc.gpsimd.tensor_copy(out=pg[:], in_=biasblk[:])
                for k in range(4):
                    rhs = xt[m][:, 128 * k + 32 * a:128 * k + 32 * a + 32]
                    for r in range(16):
                        nc.tensor.matmul(
                            out=pg[:, 32 * r:32 * r + 32],
                            lhsT=wihT[:, G4 * k + 128 * r:G4 * k + 128 * (r + 1)],
                            rhs=rhs, start=False, stop=False,
                            skip_group_check=True)
                return pg

            def emit_h(s, pg):
                for r in range(16):      # block-major: g blocks finish first
                    for k in range(4):
                        if s == 0:
                            rhs = h0T[:, 32 * k:32 * (k + 1)]
                        else:
                            c0 = M_TOK * k + 32 * (s - 1)
                            rhs = h_allT[:, c0:c0 + 32]
                        nc.tensor.matmul(
                            out=pg[:, 32 * r:32 * r + 32],
                            lhsT=whhT[:, G4 * k + 128 * r:G4 * k + 128 * (r + 1)],
                            rhs=rhs, start=False, stop=(k == 3),
                            skip_group_check=True)

            def emit_chunk(m, g0, w, wl_t, eng):
                """Logits for m-tile m, vocab cols [g0:g0+w] (within wl_t)."""
                rows = min(128, M_TOK - 128 * m)
                coff = g0 % CW
                pl = p3ps.tile([128, 512], f32, tag="pl")
                for k in range(4):
                    nc.tensor.matmul(
                        out=pl[0:rows, 0:w],
                        lhsT=h_allT[:, M_TOK * k + 128 * m:M_TOK * k + 128 * m + rows],
                        rhs=wl_t[:, CW * k + coff:CW * k + coff + w],
                        start=(k == 0), stop=(k == 3))
                stg = stp.tile([128, 512], bf16, tag="stg")
                eng.tensor_tensor(out=stg[0:rows, 0:w], in0=pl[0:rows, 0:w],
                                  in1=blin_sb[0:rows, g0:g0 + w], op=ADD)
                nc.sync.dma_start(out=out_d[128 * m:128 * m + rows, g0:g0 + w],
                                  in_=stg[0:rows, 0:w])

            # reshaped views for the strided hT write (4 chunks of 32 cols)
            hv = h_allT[:].rearrange("p (k c) -> p k c", k=4)
            ov = act_sb[:, 384:512].rearrange("p (k c) -> p k c", k=4)
            tv = th[:].rearrange("p (k c) -> p k c", k=4)

            # prologue: pre-accumulate bias+x for step 0 only (a deeper
            # prologue would sit ahead of step 0's h-MMs in the in-order PE
            # queue and delay the whole chain); the loop tops up to depth 5.
            emit_x(0)
            next_x = 1

            # emit pair schedule: (sup, m, c), consumed one per step from
            # step 4, two per step from step 16 (ready-frontier permitting).
            # sup0 pairs take priority; sup1 only after wl1 is loaded (s>=13).
            q0 = [(0, m, c) for m in range(7) for c in range(4)]
            q1 = [(1, m, c) for m in range(7) for c in range(4)]
            wl1 = None

            # ---------- recurrence ----------
            for s in range(S):
                pg = pgs.pop(s)
                emit_h(s, pg)
                # activations: g first, then i,f, then o (blocks g|i|f|o)
                nc.scalar.activation(out=act_sb[:, 0:128],
                                     in_=pg[:, 0:128], func=TANH)
                nc.scalar.activation(out=act_sb[:, 128:384],
                                     in_=pg[:, 128:384], func=SIG)
                nc.scalar.activation(out=act_sb[:, 384:512],
                                     in_=pg[:, 384:512], func=SIG)
                nc.gpsimd.tensor_tensor(out=t1[:], in0=act_sb[:, 128:256],
                                        in1=act_sb[:, 0:128], op=MUL)
                nc.gpsimd.tensor_tensor(out=t2[:], in0=act_sb[:, 256:384],
                                        in1=cT[:], op=MUL)
                nc.gpsimd.tensor_tensor(out=cT[:], in0=t1[:], in1=t2[:], op=ADD)
                nc.scalar.activation(out=th[:], in_=cT[:], func=TANH)
                nc.gpsimd.tensor_tensor(out=hv[:, :, 32 * s:32 * s + 32],
                                        in0=ov, in1=tv, op=MUL)
                # -- interleaved logits chunks (sup0 + sup1) --
                if s >= 4:
                    frontier = (s - 4) // 4
                    nem = 1 if s < 16 else 2
                    for _ in range(nem):
                        if q0 and q0[0][1] <= frontier:
                            sup, em, ec = q0.pop(0)
                            emit_chunk(em, CW * sup + EC * ec, EC, wl0,
                                       nc.vector)
                        elif s >= 13 and q1 and q1[0][1] <= frontier:
                            sup, em, ec = q1.pop(0)
                            emit_chunk(em, CW * sup + EC * ec, EC, wl1,
                                       nc.vector)
                # -- background gathers for m-tiles 2..7 --
                if s % 2 == 0 and s // 2 + 2 < NMT:
                    gather(s // 2 + 2)
                # -- pre-accumulate x-side, up to 2/step, depth <= 5 --
                for _ in range(2):
                    if next_x < S and next_x <= s + 5:
                        emit_x(next_x)
                        next_x += 1
                # -- prefetch the sup1 W_lin chunk once gathers are done --
                if s == 11:
                    wl1 = wlp.tile([128, 4 * CW], bf16, tag="wl", name="wl1")
                    for k in range(4):
                        nc.sync.dma_start(out=wl1[:, CW * k:CW * (k + 1)],
                                          in_=wlinT_d[:, V * k + CW:V * k + 2 * CW])

            # ---------- phase 3 tail ----------
            tail = (q0 + [(0, 7, c) for c in range(4)]
                    + q1 + [(1, 7, c) for c in range(4)])
            for sup in range(2, NSUP):
                tail += [(sup, m, c) for m in range(NMT) for c in range(4)]
            wl_map = {0: wl0, 1: wl1}

            def load_wl(sup):
                t = wlp.tile([128, 4 * CW], bf16, tag="wl", name=f"wl{sup}")
                for k in range(4):
                    nc.scalar.dma_start(
                        out=t[:, CW * k:CW * (k + 1)],
                        in_=wlinT_d[:, V * k + CW * sup:V * k + CW * (sup + 1)])
                wl_map[sup] = t

            load_wl(2)
            engs = [nc.vector, nc.gpsimd]
            cur_sup = 0
            for i, (sup, m, c) in enumerate(tail):
                if sup != cur_sup:
                    cur_sup = sup
                    if sup + 1 < NSUP:
                        load_wl(sup + 1)
                emit_chunk(m, CW * sup + EC * c, EC, wl_map[sup], engs[i % 2])

    nc.compile()
    return nc


def _prep_host(caps, latent, embed, W_ih, W_hh, b_ih, b_hh, W_lin, b_lin):
    import ml_dtypes
    bf = ml_dtypes.bfloat16

    caps = np.asarray(caps).astype(np.int32)
    latent = np.asarray(latent, dtype=np.float32)
    # permute gate dim to [g, i, f, o] block order
    perm = np.r_[1024:1536, 0:512, 512:1024, 1536:2048]
    W_ih_p = np.asarray(W_ih, dtype=np.float32)[perm]       # [2048, 512]
    W_hh_p = np.asarray(W_hh, dtype=np.float32)[perm]
    bias_p = (np.asarray(b_ih, dtype=np.float32)
              + np.asarray(b_hh, dtype=np.float32))[perm]

    def karrange(WT):  # [512, 2048] -> [128, 4*2048], k-chunk k at 2048k
        return np.ascontiguousarray(
            WT.reshape(4, 128, G4).transpose(1, 0, 2).reshape(128, 4 * G4))

    emb = np.ascontiguousarray(np.asarray(embed, dtype=np.float32)).astype(bf)
    wihT = karrange(W_ih_p.T).astype(bf)
    whhT = karrange(W_hh_p.T).astype(bf)
    biasblk = np.ascontiguousarray(
        np.repeat(bias_p.reshape(16, 128).transpose(1, 0)[:, :, None],
                  32, axis=2).reshape(128, 512)).astype(np.float32)
    wlinT = np.ascontiguousarray(
        np.asarray(W_lin, dtype=np.float32).T.reshape(4, 128, V)
        .transpose(1, 0, 2).reshape(128, 4 * V)).astype(bf)
    blin = np.ascontiguousarray(np.broadcast_to(
        np.asarray(b_lin, dtype=np.float32)[None, :], (128, V))).astype(bf)

    in_maps = []
    for c in range(NCORES):
        caps_sh = caps[c * BL:(c + 1) * BL]                 # [32, 32]
        tok_flat = caps_sh[:, :S].T.reshape(M_TOK)          # t-major [992]
        tok_pad = np.zeros(NMT * 128, dtype=np.int32)
        tok_pad[:M_TOK] = tok_flat
        tok = np.ascontiguousarray(tok_pad.reshape(NMT, 128).T)
        lat_sh = latent[c * BL:(c + 1) * BL]                # [32, 512]
        h0T = np.ascontiguousarray(
            lat_sh.T.reshape(4, 128, 32).transpose(1, 0, 2)
            .reshape(128, 128)).astype(bf)
        # host-transposed embeddings for m-tiles 0/1 (first 256 token rows):
        # xt layout [128, (k, j)]: E-chunk k at 128k, token col j
        x01 = np.asarray(emb)[tok_flat[:256]]               # [256, 512] bf16
        xt01 = np.ascontiguousarray(
            x01.T.reshape(4, 128, 2, 128).transpose(1, 2, 0, 3)
            .reshape(128, 1024)).astype(bf)
        in_maps.append(dict(
            emb=emb, wihT=wihT, whhT=whhT, biasblk=biasblk,
            h0T=h0T, tok=tok, wlinT=wlinT, blin=blin, xt01=xt01,
            warm=np.zeros((128, 512), dtype=np.float32).astype(bf),
        ))
    return in_maps


def kernel(caps, latent, embed, W_ih, W_hh, b_ih, b_hh, W_lin, b_lin):
    from concourse.bass_utils import run_bass_kernel_spmd

    if "nc" not in _CACHE:
        _CACHE["nc"] = _build()
    nc = _CACHE["nc"]

    in_maps = _prep_host(caps, latent, embed, W_ih, W_hh, b_ih, b_hh,
                         W_lin, b_lin)
    res = run_bass_kernel_spmd(nc, in_maps, core_ids=list(range(NCORES)))
    out = np.zeros((T, B_FULL, V), dtype=np.float32)
    for c in range(NCORES):
        shard = np.asarray(res.results[c]["out"]).astype(np.float32)
        out[1:, c * BL:(c + 1) * BL, :] = shard.reshape(S, BL, V)
    return out


# revision 34
# speedup vs baseline: 1.0056x; 1.0056x over previous
"""Teacher-forced decoder LSTM on 8 TRN2 NeuronCores.

Problem: B=256, T=32, V=10000, E=H=512 (fp32 in/out).
  step s in 0..30: x = embed[caps[:, s]]
                   gates = x@W_ih.T + h@W_hh.T + b     (i,f,g,o)
                   c = sig(f)*c + sig(i)*tanh(g); h = sig(o)*tanh(c)
                   out[s+1] = h@W_lin.T + b_lin
  out[0] = 0.  Output [T, B, V].

Sharding: data-parallel over batch, B_local=32 per core.

Key idea vs the straightforward layout: keep the whole recurrence in
TRANSPOSED space. Gates are computed as gatesT[4H, B_local] via
out[128,32] = W_chunk.T @ hT_chunk matmuls, so the PE moving dimension
is the batch (32) instead of the gate dim (512): per-step PE cost drops
~4x and the cell update produces hT directly in the layout that both
the next step's matmuls and the final logits GEMM consume - no per-step
transposes at all. All matmul operands are bf16 (1 cycle/row at any
moving size); psum accumulation stays fp32 and the cell state c is fp32.

Per step: 64 h-side MMs (N=32) on the critical path; 64 x-side MMs +
1 bias MM (N=512, via a block-indicator rhs) pre-accumulated into one
of 6 rotating psum banks several steps ahead; gate blocks are ordered
[g,i,f,o] (host-permuted weights) so tanh(g) can start early. One
500-vocab-column logits chunk is emitted per step into recurrence gaps.
Phase 3 streams W_lin.T in bf16 super-chunks; logits are stored bf16
and widened to fp32 on the host.
"""
import numpy as np

B_FULL, T, V, E, H = 256, 32, 10000, 512, 512
NCORES = 8
BL = B_FULL // NCORES          # 32 batch rows per core
S = T - 1                      # 31 recurrent steps
M_TOK = S * BL                 # 992 token rows per core (t-major)
NMT = (M_TOK + 127) // 128     # 8 m-tiles (last has 96 rows)
G4 = 4 * H                     # 2048 gate dims
CW = 2000                      # vocab super-chunk width
NSUP = V // CW                 # 5 super-chunks
EC = 500                       # emit chunk width (CW // 4)

_CACHE = {}


def _build():
    import concourse.bacc as bacc
    import concourse.mybir as mybir
    from concourse.tile import TileContext
    import concourse.bass as bass

    f32 = mybir.dt.float32
    bf16 = mybir.dt.bfloat16
    i32 = mybir.dt.int32
    SIG = mybir.ActivationFunctionType.Sigmoid
    TANH = mybir.ActivationFunctionType.Tanh
    ADD = mybir.AluOpType.add
    MUL = mybir.AluOpType.mult

    nc = bacc.Bacc()

    emb_d = nc.dram_tensor("emb", [V, E], bf16, kind="ExternalInput")
    # wihT/whhT pre-arranged on host to [128, 4k x 2048]: k-chunk k at free
    # [2048k:2048(k+1)], gate blocks inside permuted to [g,i,f,o] order.
    wihT_d = nc.dram_tensor("wihT", [128, 4 * G4], bf16, kind="ExternalInput")
    whhT_d = nc.dram_tensor("whhT", [128, 4 * G4], bf16, kind="ExternalInput")
    biasblk_d = nc.dram_tensor("biasblk", [128, 512], f32, kind="ExternalInput")
    h0T_d = nc.dram_tensor("h0T", [128, 128], bf16, kind="ExternalInput")
    tok_d = nc.dram_tensor("tok", [128, NMT], i32, kind="ExternalInput")
    # host-transposed embeddings for m-tiles 0/1 (startup critical path);
    # m-tiles 2..7 are gathered+transposed on device during the recurrence
    xt01_d = nc.dram_tensor("xt01", [128, 1024], bf16, kind="ExternalInput")
    warm_d = nc.dram_tensor("warm", [128, 512], bf16, kind="ExternalInput")
    # wlinT pre-arranged to [128, 4k x 10000]: k-chunk k at [10000k:...]
    wlinT_d = nc.dram_tensor("wlinT", [128, 4 * V], bf16, kind="ExternalInput")
    blin_d = nc.dram_tensor("blin", [128, V], bf16, kind="ExternalInput")
    out_d = nc.dram_tensor("out", [M_TOK, V], bf16, kind="ExternalOutput")

    with TileContext(nc) as tc:
        with tc.tile_pool(name="const", bufs=1) as cp, \
             tc.tile_pool(name="state", bufs=1) as st, \
             tc.tile_pool(name="xst", bufs=2) as xst, \
             tc.tile_pool(name="wlp", bufs=2) as wlp, \
             tc.tile_pool(name="stg", bufs=4) as stp, \
             tc.tile_pool(name="rps", bufs=6, space="PSUM") as rps, \
             tc.tile_pool(name="p3ps", bufs=2, space="PSUM") as p3ps:

            # ---------- constant loads, spread across queues ----------
            # Startup critical path: x(0) needs xt01 + bias16/sel16 + wihT;
            # h(0) additionally needs whhT + h0T. Ws are split in quarters
            # across all 4 DMA-capable queues so each is resident ~2us after
            # its loads start.
            QW = G4  # quarter width of the [128, 4*G4] layout
            wihT = cp.tile([128, 4 * G4], bf16, tag="wihT")
            whhT = cp.tile([128, 4 * G4], bf16, tag="whhT")
            tok_sb = cp.tile([128, NMT], i32, tag="tok_sb")
            biasblk = cp.tile([128, 512], f32, tag="biasblk")
            h0T = cp.tile([128, 128], bf16, tag="h0T")

            # xt[m]: transposed gathered embeddings for m-tile m,
            # E-chunk k at [128k:128(k+1)], token j at col j (4 steps x 32).
            xt = [st.tile([128, 512], bf16, tag=f"xt{m}", name=f"xt{m}")
                  for m in range(NMT)]

            def wq(w_sb, w_d, q, eng):
                eng.dma_start(out=w_sb[:, QW * q:QW * (q + 1)],
                              in_=w_d[:, QW * q:QW * (q + 1)])

            # SP queue: a small warm tile first, then xt for m-tiles 0/1
            warmm = cp.tile([128, 512], bf16, tag="warmm")
            nc.sync.dma_start(out=warmm[:], in_=warm_d[:])
            nc.sync.dma_start(out=xt[0][:], in_=xt01_d[:, 0:512])
            nc.sync.dma_start(out=xt[1][:], in_=xt01_d[:, 512:1024])
            # (PE p-state warmup tried here: net-negative, removed)
            wq(wihT, wihT_d, 0, nc.sync)
            wq(whhT, whhT_d, 0, nc.sync)
            wq(wihT, wihT_d, 3, nc.sync)
            # ACT queue
            wq(wihT, wihT_d, 1, nc.scalar)
            wq(whhT, whhT_d, 1, nc.scalar)
            wq(whhT, whhT_d, 3, nc.scalar)
            # Pool queue
            nc.gpsimd.dma_start(out=tok_sb[:], in_=tok_d[:])
            nc.gpsimd.dma_start(out=biasblk[:], in_=biasblk_d[:])
            nc.gpsimd.dma_start(out=h0T[:], in_=h0T_d[:])
            wq(wihT, wihT_d, 2, nc.gpsimd)
            wq(whhT, whhT_d, 2, nc.gpsimd)
            # lower-priority loads (behind the startup chain)
            wl0 = wlp.tile([128, 4 * CW], bf16, tag="wl", name="wl0")
            for k in range(4):
                nc.scalar.dma_start(out=wl0[:, CW * k:CW * (k + 1)],
                                    in_=wlinT_d[:, V * k:V * k + CW])

            # ---------- state ----------
            # h_allT: transposed hidden states, chunk k at [992k:992(k+1)],
            # step s at cols 32s within each chunk. bf16; rhs of recurrence
            # MMs and lhsT of phase-3 MMs.
            h_allT = st.tile([128, 4 * M_TOK], bf16, tag="h_allT")
            cT = st.tile([128, 128], f32, tag="cT")
            nc.vector.memset(cT[:], 0.0)
            act_sb = st.tile([128, 512], f32, tag="act_sb")  # g|i|f|o blocks
            t1 = st.tile([128, 128], f32, tag="t1")
            t2 = st.tile([128, 128], f32, tag="t2")
            th = st.tile([128, 128], f32, tag="th")

            def gather(m):
                rows = min(128, M_TOK - 128 * m)
                gx = xst.tile([128, 512], bf16, tag="gx", name=f"gx{m}")
                nc.gpsimd.indirect_dma_start(
                    out=gx[0:rows, :], out_offset=None, in_=emb_d[:],
                    in_offset=bass.IndirectOffsetOnAxis(
                        ap=tok_sb[0:rows, m:m + 1], axis=0))
                # single chunked-transpose DMA: out[p, k, j] = gx[j, 128k+p]
                nc.sync.dma_start_transpose(
                    out=xt[m][:].rearrange("p (k j) -> p k j", k=4)[:, :, 0:rows],
                    in_=gx[0:rows, :])

            blin_sb = cp.tile([128, V], bf16, tag="blin_sb")
            nc.sync.dma_start(out=blin_sb[:], in_=blin_d[:])

            # ---------- recurrence helpers ----------
            pgs = {}

            def emit_x(s):
                """Bias init + x-side gate MMs for step s into a fresh psum
                bank. The bias is written by a Pool copy (not a PE matmul);
                all MMs then accumulate with start=False. Safe because every
                bank cycle writes all 512 columns, so no pending-zero bits
                survive from the previous user of the bank."""
                m, a = divmod(s, 4)
                pg = rps.tile([128, 512], f32, tag="pg", name=f"pg{s}")
                pgs[s] = pg
                nc.gpsimd.tensor_copy(out=pg[:], in_=biasblk[:])
                for k in range(4):
                    rhs = xt[m][:, 128 * k + 32 * a:128 * k + 32 * a + 32]
                    for r in range(16):
                        nc.tensor.matmul(
                            out=pg[:, 32 * r:32 * r + 32],
                            lhsT=wihT[:, G4 * k + 128 * r:G4 * k + 128 * (r + 1)],
                            rhs=rhs, start=False, stop=False,
                            skip_group_check=True)
                return pg

            def emit_h(s, pg):
                for r in range(16):      # block-major: g blocks finish first
                    for k in range(4):
                        if s == 0:
                            rhs = h0T[:, 32 * k:32 * (k + 1)]
                        else:
                            c0 = M_TOK * k + 32 * (s - 1)
                            rhs = h_allT[:, c0:c0 + 32]
                        nc.tensor.matmul(
                            out=pg[:, 32 * r:32 * r + 32],
                            lhsT=whhT[:, G4 * k + 128 * r:G4 * k + 128 * (r + 1)],
                            rhs=rhs, start=False, stop=(k == 3),
                            skip_group_check=True)

            def emit_chunk(m, g0, w, wl_t, eng):
                """Logits for m-tile m, vocab cols [g0:g0+w] (within wl_t)."""
                rows = min(128, M_TOK - 128 * m)
                coff = g0 % CW
                pl = p3ps.tile([128, 512], f32, tag="pl")
                for k in range(4):
                    nc.tensor.matmul(
                        out=pl[0:rows, 0:w],
                        lhsT=h_allT[:, M_TOK * k + 128 * m:M_TOK * k + 128 * m + rows],
                        rhs=wl_t[:, CW * k + coff:CW * k + coff + w],
                        start=(k == 0), stop=(k == 3))
                stg = stp.tile([128, 512], bf16, tag="stg")
                eng.tensor_tensor(out=stg[0:rows, 0:w], in0=pl[0:rows, 0:w],
                                  in1=blin_sb[0:rows, g0:g0 + w], op=ADD)
                nc.sync.dma_start(out=out_d[128 * m:128 * m + rows, g0:g0 + w],
                                  in_=stg[0:rows, 0:w])

            # reshaped views for the strided hT write (4 chunks of 32 cols)
            hv = h_allT[:].rearrange("p (k c) -> p k c", k=4)
            ov = act_sb[:, 384:512].rearrange("p (k c) -> p k c", k=4)
            tv = th[:].rearrange("p (k c) -> p k c", k=4)

            # prologue: pre-accumulate bias+x for step 0 only (a deeper
            # prologue would sit ahead of step 0's h-MMs in the in-order PE
            # queue and delay the whole chain); the loop tops up to depth 5.
            emit_x(0)
            next_x = 1

            # emit pair schedule: (sup, m, c), consumed one per step from
            # step 4, two per step from step 16 (ready-frontier permitting).
            # sup0 pairs take priority; sup1 only after wl1 is loaded (s>=13).
            q0 = [(0, m, c) for m in range(7) for c in range(4)]
            q1 = [(1, m, c) for m in range(7) for c in range(4)]
            wl1 = None

            # ---------- recurrence ----------
            for s in range(S):
                pg = pgs.pop(s)
                emit_h(s, pg)
                # activations: g first, then i,f, then o (blocks g|i|f|o)
                nc.scalar.activation(out=act_sb[:, 0:128],
                                     in_=pg[:, 0:128], func=TANH)
                nc.scalar.activation(out=act_sb[:, 128:384],
                                     in_=pg[:, 128:384], func=SIG)
                nc.scalar.activation(out=act_sb[:, 384:512],
                                     in_=pg[:, 384:512], func=SIG)
                nc.gpsimd.tensor_tensor(out=t1[:], in0=act_sb[:, 128:256],
                                        in1=act_sb[:, 0:128], op=MUL)
                nc.gpsimd.tensor_tensor(out=t2[:], in0=act_sb[:, 256:384],
                                        in1=cT[:], op=MUL)
                nc.gpsimd.tensor_tensor(out=cT[:], in0=t1[:], in1=t2[:], op=ADD)
                nc.scalar.activation(out=th[:], in_=cT[:], func=TANH)
                nc.gpsimd.tensor_tensor(out=hv[:, :, 32 * s:32 * s + 32],
                                        in0=ov, in1=tv, op=MUL)
                # -- interleaved logits chunks (sup0 + sup1) --
                if s >= 4:
                    frontier = (s - 4) // 4
                    nem = 1 if s < 16 else 2
                    for _ in range(nem):
                        if q0 and q0[0][1] <= frontier:
                            sup, em, ec = q0.pop(0)
                            emit_chunk(em, CW * sup + EC * ec, EC, wl0,
                                       nc.vector)
                        elif s >= 13 and q1 and q1[0][1] <= frontier:
                            sup, em, ec = q1.pop(0)
                            emit_chunk(em, CW * sup + EC * ec, EC, wl1,
                                       nc.vector)
                # -- background gathers for m-tiles 2..7 --
                if s % 2 == 0 and s // 2 + 2 < NMT:
                    gather(s // 2 + 2)
                # -- pre-accumulate x-side, up to 2/step, depth <= 5 --
                for _ in range(2):
                    if next_x < S and next_x <= s + 5:
                        emit_x(next_x)
                        next_x += 1
                # -- prefetch the sup1 W_lin chunk once gathers are done --
                if s == 11:
                    wl1 = wlp.tile([128, 4 * CW], bf16, tag="wl", name="wl1")
                    for k in range(4):
                        nc.sync.dma_start(out=wl1[:, CW * k:CW * (k + 1)],
                                          in_=wlinT_d[:, V * k + CW:V * k + 2 * CW])

            # ---------- phase 3 tail ----------
            tail = (q0 + [(0, 7, c) for c in range(4)]
                    + q1 + [(1, 7, c) for c in range(4)])
            for sup in range(2, NSUP):
                tail += [(sup, m, c) for m in range(NMT) for c in range(4)]
            wl_map = {0: wl0, 1: wl1}

            def load_wl(sup):
                t = wlp.tile([128, 4 * CW], bf16, tag="wl", name=f"wl{sup}")
                for k in range(4):
                    nc.scalar.dma_start(
                        out=t[:, CW * k:CW * (k + 1)],
                        in_=wlinT_d[:, V * k + CW * sup:V * k + CW * (sup + 1)])
                wl_map[sup] = t

            load_wl(2)
            engs = [nc.vector, nc.gpsimd]
            cur_sup = 0
            for i, (sup, m, c) in enumerate(tail):
                if sup != cur_sup:
                    cur_sup = sup
                    if sup + 1 < NSUP:
                        load_wl(sup + 1)
                emit_chunk(m, CW * sup + EC * c, EC, wl_map[sup], engs[i % 2])

    nc.compile()
    return nc


def _prep_host(caps, latent, embed, W_ih, W_hh, b_ih, b_hh, W_lin, b_lin):
    import ml_dtypes
    bf = ml_dtypes.bfloat16

    caps = np.asarray(caps).astype(np.int32)
    latent = np.asarray(latent, dtype=np.float32)
    # permute gate dim to [g, i, f, o] block order
    perm = np.r_[1024:1536, 0:512, 512:1024, 1536:2048]
    W_ih_p = np.asarray(W_ih, dtype=np.float32)[perm]       # [2048, 512]
    W_hh_p = np.asarray(W_hh, dtype=np.float32)[perm]
    bias_p = (np.asarray(b_ih, dtype=np.float32)
              + np.asarray(b_hh, dtype=np.float32))[perm]

    def karrange(WT):  # [512, 2048] -> [128, 4*2048], k-chunk k at 2048k
        return np.ascontiguousarray(
            WT.reshape(4, 128, G4).transpose(1, 0, 2).reshape(128, 4 * G4))

    emb = np.ascontiguousarray(np.asarray(embed, dtype=np.float32)).astype(bf)
    wihT = karrange(W_ih_p.T).astype(bf)
    whhT = karrange(W_hh_p.T).astype(bf)
    biasblk = np.ascontiguousarray(
        np.repeat(bias_p.reshape(16, 128).transpose(1, 0)[:, :, None],
                  32, axis=2).reshape(128, 512)).astype(np.float32)
    wlinT = np.ascontiguousarray(
        np.asarray(W_lin, dtype=np.float32).T.reshape(4, 128, V)
        .transpose(1, 0, 2).reshape(128, 4 * V)).astype(bf)
    blin = np.ascontiguousarray(np.broadcast_to(
        np.asarray(b_lin, dtype=np.float32)[None, :], (128, V))).astype(bf)

    in_maps = []
    for c in range(NCORES):
        caps_sh = caps[c * BL:(c + 1) * BL]                 # [32, 32]
        tok_flat = caps_sh[:, :S].T.reshape(M_TOK)          # t-major [992]
        tok_pad = np.zeros(NMT * 128, dtype=np.int32)
        tok_pad[:M_TOK] = tok_flat
        tok = np.ascontiguousarray(tok_pad.reshape(NMT, 128).T)
        lat_sh = latent[c * BL:(c + 1) * BL]                # [32, 512]
        h0T = np.ascontiguousarray(
            lat_sh.T.reshape(4, 128, 32).transpose(1, 0, 2)
            .reshape(128, 128)).astype(bf)
        # host-transposed embeddings for m-tiles 0/1 (first 256 token rows):
        # xt layout [128, (k, j)]: E-chunk k at 128k, token col j
        x01 = np.asarray(emb)[tok_flat[:256]]               # [256, 512] bf16
        xt01 = np.ascontiguousarray(
            x01.T.reshape(4, 128, 2, 128).transpose(1, 2, 0, 3)
            .reshape(128, 1024)).astype(bf)
        in_maps.append(dict(
            emb=emb, wihT=wihT, whhT=whhT, biasblk=biasblk,
            h0T=h0T, tok=tok, wlinT=wlinT, blin=blin, xt01=xt01,
            warm=np.zeros((128, 512), dtype=np.float32).astype(bf),
        ))
    return in_maps


def kernel(caps, latent, embed, W_ih, W_hh, b_ih, b_hh, W_lin, b_lin):
    from concourse.bass_utils import run_bass_kernel_spmd

    if "nc" not in _CACHE:
        _CACHE["nc"] = _build()
    nc = _CACHE["nc"]

    in_maps = _prep_host(caps, latent, embed, W_ih, W_hh, b_ih, b_hh,
                         W_lin, b_lin)
    res = run_bass_kernel_spmd(nc, in_maps, core_ids=list(range(NCORES)))
    out = np.zeros((T, B_FULL, V), dtype=np.float32)
    for c in range(NCORES):
        shard = np.asarray(res.results[c]["out"]).astype(np.float32)
        out[1:, c * BL:(c + 1) * BL, :] = shard.reshape(S, BL, V)
    return out
